# revision 1
# baseline (speedup 1.0000x reference)
"""BTSPAttention Trainium2 kernel for 8 NeuronCores (self-contained).

Usage: kernel(**inputs) -> np.ndarray  (full [2,2048,1024] float32 output)


Sharding: 8 cores = 2 batches x 4 head-groups (4 heads each).
Per-core dataflow (everything keeps the query/time axis in the free dim):
  QT/KT [256,2048] = W_local @ x^T (+bias on ACT evac)   (fp32r matmuls)
  V     [2048,256] = x @ Wv_local^T                      (x^T-block stationary)
  per head: scoresT[k,q] = KT_h^T.T @ QT_h  (PSUM, fp32)
            P = exp(0.125*scoresT) * E      (ACT exp -> bf16, DVE mul by
                                             replicated Toeplitz table E)
            ctxT[65,q] accum = [V_h|1]^T @ P (bf16 matmul; row 64 = softmax sum)
            normalize via DVE recip + gpsimd partition-broadcast
  out: the reference's faithful-torch 5-D transpose scrambles (B,H) into
       output rows; per head the output rows are DISJOINT:
       out[h//8, (h%8)*256 + b*128 + tc, :] = g_h @ Wo.T
       with g_h = ctxn_h.reshape(128,1024). Done as 16 K=64 matmuls with a
       stride-16 AP on ctxn^T (bf16). No cross-core reduction needed.
Host folds: is_gate dropped (softmax shift-invariance); bv and bo applied
exactly on the host after gather.
"""

import numpy as np
import ml_dtypes

import sys as _sys
if '/opt/trn_rl_repo' not in _sys.path:
    _sys.path.insert(0, '/opt/trn_rl_repo')


import concourse.bass as bass
import concourse.tile as tile
from concourse import bacc
from concourse import mybir

F32 = mybir.dt.float32
F32R = mybir.dt.float32r
BF16 = mybir.dt.bfloat16
AF = mybir.ActivationFunctionType

T = 2048
D = 1024
HD = 64
TB_LEN = 500
NKC = 16   # k chunks of 128
NDC = 8    # D chunks of 128

def host_prep(inputs):
    """Returns (in_maps for 8 cores, postprocess-closure)."""
    x = np.asarray(inputs["x"], np.float32)
    Wq = np.asarray(inputs["Wq"], np.float32)
    Wk = np.asarray(inputs["Wk"], np.float32)
    Wv = np.asarray(inputs["Wv"], np.float32)
    Wo = np.asarray(inputs["Wo"], np.float32)
    bq = np.asarray(inputs["bq"], np.float32)
    bk = np.asarray(inputs["bk"], np.float32)
    bv = np.asarray(inputs["bv"], np.float32)
    bo = np.asarray(inputs["bo"], np.float32)
    et = float(np.asarray(inputs["et_gate"], np.float32).reshape(()))
    tb = np.asarray(inputs["time_bias"], np.float32).reshape(-1)
    assert tb.shape == (TB_LEN,)

    sig = 1.0 / (1.0 + np.exp(-et))
    idx = np.clip(np.arange(T)[:, None] - np.arange(T)[None, :] + TB_LEN // 2,
                  0, TB_LEN - 1)              # [k, q]
    E = np.exp(np.float32(sig) * tb[idx]).astype(np.float32)
    eb = np.ascontiguousarray(
        E.reshape(NKC, 128, T).transpose(1, 0, 2)
    ).astype(ml_dtypes.bfloat16)              # [128, 16, 2048]

    # wog[j + 64*(tf%2) ... ] layout: wog[p, tf, do] with p = j (64 partitions)
    wg = np.ascontiguousarray(Wo.T.reshape(16, 64, D).transpose(1, 0, 2))  # [j, tf, do]
    wog = wg.astype(ml_dtypes.bfloat16)       # [64, 16, 1024]

    def chunk_w(Wl):  # Wl [256, 1024] -> [128, 8, 256]: [p, c, m] = Wl[m, c*128+p]
        return np.ascontiguousarray(Wl.T.reshape(NDC, 128, 256).transpose(1, 0, 2))

    in_maps = []
    for core in range(8):
        b, hg = core // 4, core % 4
        sl = slice(hg * 256, (hg + 1) * 256)
        bqk = np.stack([bq[sl][:128], bq[sl][128:],
                        bk[sl][:128], bk[sl][128:]], axis=1)  # [128, 4]
        in_maps.append({
            "xT": np.ascontiguousarray(x[b].T),
            "wq": chunk_w(Wq[sl]),
            "wk": chunk_w(Wk[sl]),
            "wv": chunk_w(Wv[sl]),
            "wog": wog,
            "bqk": np.ascontiguousarray(bqk, np.float32),
            "ones": np.ones((128, 64), np.float32),
            "eb": eb,
        })

    corr = np.einsum("hj,jfd->hd", bv.reshape(16, HD), wg).astype(np.float32)  # per global head

    def post(results):
        out = np.empty((2, T, D), np.float32)
        for core in range(8):
            b, hg = core // 4, core % 4
            yc = results[core]["y"]  # [512, 1024]
            for hl in range(4):
                h = hg * 4 + hl
                rows = (h % 8) * 256 + b * 128
                out[h // 8, rows:rows + 128, :] = (
                    yc[hl * 128:(hl + 1) * 128] + corr[h][None, :] + bo[None, :]
                )
        return out

    return in_maps, post


def expected_core(inputs, core):
    """Numpy model of one core's device output (for sim checks)."""
    m, _ = host_prep(inputs)
    im = m[core]
    xT = im["xT"]; eb = np.asarray(im["eb"], np.float32)
    E = eb.transpose(1, 0, 2).reshape(T, T)
    y = np.zeros((512, 1024), np.float32)
    wq = im["wq"]; wk = im["wk"]; wv = im["wv"]; bqk = im["bqk"]
    Wq_l = np.concatenate([wq[:, c, :] for c in range(NDC)], axis=0)  # [1024, 256] = Wl.T
    Wk_l = np.concatenate([wk[:, c, :] for c in range(NDC)], axis=0)
    Wv_l = np.concatenate([wv[:, c, :] for c in range(NDC)], axis=0)
    QT = Wq_l.T @ xT + np.concatenate([bqk[:, 0], bqk[:, 1]])[:, None]
    KT = Wk_l.T @ xT + np.concatenate([bqk[:, 2], bqk[:, 3]])[:, None]
    V = xT.T @ Wv_l
    wog = np.asarray(im["wog"], np.float32)  # [64, 16, 1024]
    for hl in range(4):
        qh = QT[hl * 64:(hl + 1) * 64]
        kh = KT[hl * 64:(hl + 1) * 64]
        P = np.exp(0.125 * (kh.T @ qh)) * E
        c = (V[:, hl * 64:(hl + 1) * 64].T @ P) / P.sum(axis=0)[None, :]  # [64, q]
        cn = c.astype(ml_dtypes.bfloat16).astype(np.float32)
        # y[tc, do] = sum_{tf,j} cn[j, 16tc+tf] * wog[j, tf, do]
        g = cn.reshape(64, 128, 16)
        y[hl * 128:(hl + 1) * 128] = np.einsum("jcf,jfd->cd", g, wog)
    return y


def build_program(repeats=1):
    nc = bacc.Bacc("TRN2", target_bir_lowering=False, debug=False,
                   dynamic_dma_scratch_size=4096)
    xT = nc.dram_tensor("xT", [D, T], F32R, kind="ExternalInput").ap()
    wq_d = nc.dram_tensor("wq", [128, NDC, 256], F32R, kind="ExternalInput").ap()
    wk_d = nc.dram_tensor("wk", [128, NDC, 256], F32R, kind="ExternalInput").ap()
    wv_d = nc.dram_tensor("wv", [128, NDC, 256], F32R, kind="ExternalInput").ap()
    wog_d = nc.dram_tensor("wog", [64, 16, D], BF16, kind="ExternalInput").ap()
    bqk_d = nc.dram_tensor("bqk", [128, 4], F32, kind="ExternalInput").ap()
    ones_d = nc.dram_tensor("ones", [128, 64], F32R, kind="ExternalInput").ap()
    eb_d = nc.dram_tensor("eb", [128, NKC, T], BF16, kind="ExternalInput").ap()
    y_d = nc.dram_tensor("y", [512, D], F32, kind="ExternalOutput").ap()

    with tile.TileContext(nc) as tc:
        with (
            tc.tile_pool(name="const", bufs=1) as const,
            tc.tile_pool(name="persist", bufs=1) as persist,
            tc.tile_pool(name="xp", bufs=2) as xp,
            tc.tile_pool(name="pp", bufs=5) as pp,
            tc.tile_pool(name="ctxnp", bufs=2) as ctxnp,
            tc.tile_pool(name="rbp", bufs=1) as rbp,
            tc.tile_pool(name="yevac", bufs=2) as yevac,
            tc.tile_pool(name="scps", bufs=2, space="PSUM") as scps,
            tc.tile_pool(name="ctxps", bufs=4, space="PSUM") as ctxps,
        ):
            # ---- constants ----
            wq = const.tile([128, NDC, 256], F32R, tag="wq")
            wk = const.tile([128, NDC, 256], F32R, tag="wk")
            wv = const.tile([128, NDC, 256], F32R, tag="wv")
            wog = const.tile([64, 16, D], BF16, tag="wog")
            bqk = const.tile([128, 4], F32, tag="bqk")
            ones_r = const.tile([128, 64], F32R, tag="ones_r")
            eb = const.tile([128, NKC, T], BF16, tag="eb")
            nc.sync.dma_start(wq[:], wq_d[:])
            nc.sync.dma_start(wk[:], wk_d[:])
            nc.sync.dma_start(wv[:], wv_d[:])
            nc.sync.dma_start(wog[:], wog_d[:])
            nc.sync.dma_start(bqk[:], bqk_d[:])
            nc.sync.dma_start(ones_r[:], ones_d[:])
            for c in range(NKC):
                nc.sync.dma_start(eb[:, c, :], eb_d[:, c, :])

            for _r in range(repeats):
                qT = [persist.tile([128, T], F32R, tag=f"qT{i}", name=f"qT{i}_{_r}") for i in range(2)]
                kT = [persist.tile([128, T], F32R, tag=f"kT{i}", name=f"kT{i}_{_r}") for i in range(2)]
                v_sb = persist.tile([128, NKC, 4, 65], BF16, tag="v_sb")
                nc.vector.memset(v_sb[:], 1.0)

                # ---- QKV projections ----
                for s in range(4):  # q-slices of 512
                    q_ps = [ctxps.tile([128, 512], F32, tag="ctx", name=f"qps{i}_{_r}_{s}") for i in range(2)]
                    k_ps = [ctxps.tile([128, 512], F32, tag="ctx", name=f"kps{i}_{_r}_{s}") for i in range(2)]
                    # each 256-wide V accumulation group gets its own 2KB bank
                    v_ps = [scps.tile([128, 2, 512], F32, tag="sc", name=f"vps{i}_{_r}_{s}") for i in range(2)]
                    for c in range(NDC):
                        xc = xp.tile([128, 512], F32R, tag="xc")
                        nc.sync.dma_start(
                            xc[:], xT[c * 128:(c + 1) * 128, s * 512:(s + 1) * 512])
                        st, sp = (c == 0), (c == NDC - 1)
                        xr = xc[:]
                        for hp in range(2):
                            nc.tensor.matmul(
                                q_ps[hp][:],
                                wq[:, c, hp * 128:(hp + 1) * 128],
                                xr, start=st, stop=sp)
                            nc.tensor.matmul(
                                k_ps[hp][:],
                                wk[:, c, hp * 128:(hp + 1) * 128],
                                xr, start=st, stop=sp)
                        for tb in range(4):
                            nc.tensor.matmul(
                                v_ps[tb // 2][:, tb % 2, 0:256],
                                xc[:, tb * 128:(tb + 1) * 128],
                                wv[:, c, :], start=st, stop=sp)
                    # evacuate
                    for hp in range(2):
                        nc.scalar.activation(
                            qT[hp][:, s * 512:(s + 1) * 512], q_ps[hp][:],
                            AF.Identity, bias=bqk[:, hp:hp + 1])
                        nc.scalar.activation(
                            kT[hp][:, s * 512:(s + 1) * 512], k_ps[hp][:],
                            AF.Identity, bias=bqk[:, 2 + hp:3 + hp])
                    for tb in range(4):
                        kc = s * 4 + tb
                        nc.vector.tensor_copy(
                            v_sb[:, kc, :, 0:64],
                            v_ps[tb // 2][:, tb % 2, 0:256].rearrange(
                                "p (h j) -> p h j", h=4))

                # ---- attention + per-head output projection ----
                for hl in range(4):
                    hp, off = hl // 2, (hl % 2) * 64
                    ctx_ps = [ctxps.tile([65, 512], F32, tag="ctx", name=f"cps{i}_{_r}_{hl}") for i in range(4)]

                    def emit_av(cc, pts, hl=hl, ctx_ps=ctx_ps):
                        for s2 in range(2):
                            pm = pts.pop((cc, s2))
                            for j in range(2):
                                nc.tensor.matmul(
                                    ctx_ps[2 * s2 + j][:],
                                    v_sb[:, cc, hl, :],
                                    pm[:, j * 512:(j + 1) * 512],
                                    start=(cc == 0), stop=(cc == NKC - 1))

                    pts = {}
                    for c in range(NKC):
                        lhs_k = kT[hp][off:off + 64, c * 128:(c + 1) * 128]
                        for s2 in range(2):
                            sc = scps.tile([128, 1024], F32, tag="sc")
                            for j in range(2):
                                nc.tensor.matmul(
                                    sc[:, j * 512:(j + 1) * 512], lhs_k,
                                    qT[hp][off:off + 64,
                                           s2 * 1024 + j * 512:s2 * 1024 + (j + 1) * 512],
                                    start=True, stop=True)
                            p_t = pp.tile([128, 1024], BF16, tag="p")
                            nc.scalar.activation(p_t[:], sc[:], AF.Exp, scale=0.125)
                            nc.vector.tensor_mul(
                                p_t[:], p_t[:],
                                eb[:, c, s2 * 1024:(s2 + 1) * 1024])
                            pts[(c, s2)] = p_t
                        if c >= 1:
                            emit_av(c - 1, pts)
                    emit_av(NKC - 1, pts)

                    ctxn = ctxnp.tile([64, T], BF16, tag="ctxn",
                                      name=f"ctxn_{_r}_{hl}")
                    for si in range(4):
                        rb = rbp.tile([65, 512], F32R, tag="rb",
                                      name=f"rb{si}_{_r}_{hl}")
                        with nc.allow_low_precision(reason="f32r recip"):
                            nc.vector.reciprocal(rb[64:65, :], ctx_ps[si][64:65, :])
                        bc_ps = scps.tile([64, 512], F32, tag="sc",
                                          name=f"bcps_{_r}_{hl}_{si}")
                        nc.tensor.matmul(bc_ps[:], ones_r[64:65, 0:64],
                                         rb[64:65, :], start=True, stop=True)
                        bc_sb = rbp.tile([64, 512], F32, tag="bc",
                                         name=f"bcsb_{_r}_{hl}_{si}")
                        nc.scalar.activation(bc_sb[:], bc_ps[:], AF.Copy)
                        nc.vector.tensor_mul(
                            ctxn[:, si * 512:(si + 1) * 512],
                            ctx_ps[si][0:64, :], bc_sb[:])
                    ctxr = ctxn.rearrange("p (tc tf) -> p tf tc", tf=16)
                    y_ps = [ctxps.tile([128, 512], F32, tag="ctx",
                                       name=f"yps{i}_{_r}_{hl}") for i in range(2)]
                    for tf in range(16):
                        for ds in range(2):
                            nc.tensor.matmul(
                                y_ps[ds][:], ctxr[:, tf, :],
                                wog[:, tf, ds * 512:(ds + 1) * 512],
                                start=(tf == 0), stop=(tf == 15))
                    for ds in range(2):
                        ysb = yevac.tile([128, 512], F32, tag="y",
                                         name=f"ysb{ds}_{_r}_{hl}")
                        if ds == 0:
                            nc.vector.tensor_copy(ysb[:], y_ps[ds][:])
                        else:
                            nc.scalar.activation(ysb[:], y_ps[ds][:], AF.Copy)
                        nc.sync.dma_start(
                            y_d[hl * 128:(hl + 1) * 128, ds * 512:(ds + 1) * 512],
                            ysb[:])
    nc.compile()
    return nc


import sys as _sys
if '/opt/trn_rl_repo' not in _sys.path:
    _sys.path.insert(0, '/opt/trn_rl_repo')

_PROGRAM_CACHE = {}


def _get_program(repeats=1):
    if repeats not in _PROGRAM_CACHE:
        _PROGRAM_CACHE[repeats] = build_program(repeats=repeats)
    return _PROGRAM_CACHE[repeats]


def kernel(**inputs):
    from concourse.bass_utils import run_bass_kernel_spmd
    in_maps, post = host_prep(inputs)
    nc = _get_program(repeats=1)
    res = run_bass_kernel_spmd(nc, in_maps, list(range(8)))
    return post(res.results)

